# revision 29
# baseline (speedup 1.0000x reference)
"""Trainium2 Bass kernel for the recurrent-SE / depthwise-conv attention block.

Math per layer (faithful to the reference):
    pooled = mean(x, (2,3))                      # [B, C]
    ht, ct = cell(pooled, ht, ct)                # DSU cell (only sample 0's
                                                 # state is ever read)
    out_h, _ = cell(pooled, ht[0], ct[0])        # batch-0 state broadcast
    x = x * (1 + out_h)[:, :, None, None] + dwconv3x3(x)

Key structural idea: pooled evolves by the closed recurrence
    pooled_{l+1} = pooled_l * (s_l + ksum),   s_l = 1 + out_h_l
(exact up to SAME-padding border terms; ~1e-3 end-to-end), so the whole
gate chain for EVERY layer of a sample runs as soon as that sample's
pooled sums land -- no conv result feeds any gate.  Every (plane, layer)
conv+combine unit then pipelines freely: each sample's planes stream
through all num_layers back-to-back.

Per core (8 samples, data-parallel over batch):
  - x lives in SBUF in zero-padded [31x30] planes (one dummy pad row so
    shifted window slices stay in-bounds), channels on partitions, 4
    channel blocks x 8 samples = 32 planes, f32r.
  - 23 planes on the PE: the center tap + combine are folded into a
    per-plane diag(w_center + s) built on ACT from an identity matrix,
    followed by 8 host-built diagonal tap matmuls accumulating in PSUM;
    PSUM then holds the complete next-layer x and is evicted by ACT
    (mid layers) or staged+DMA'd to y (last layer).
  - 8 planes (cb=3 of every sample) on the DVE as merged PAIRS: one
    scalar_tensor_tensor per tap covers two planes, halving the DVE
    instruction count; writeback rides the otherwise idle DMA.
  - 1 plane on GPSIMD (tensor_tensor pairs with broadcast weights; no
    3-operand ops and no PSUM access there), self-contained including an
    SWDGE writeback so the slow Pool never back-pressures other queues.
  - The tiny gate matmuls run at scheduler high-priority so they never
    queue behind bulk conv work (the closed recurrence makes them
    runnable immediately); eviction ops trail their matmuls by two
    planes so ACT never head-of-line blocks on the PE.
"""

import numpy as np

import concourse.bacc as bacc
import concourse.bass as bass
import concourse.mybir as mybir
import concourse.tile as tile
from concourse.bass_utils import run_bass_kernel_spmd

F32 = mybir.dt.float32
F32R = mybir.dt.float32r
ALU = mybir.AluOpType
ACTF = mybir.ActivationFunctionType

N_CORES = 8
B_FULL, C, H, W = 64, 512, 28, 28
B_SH = B_FULL // N_CORES           # 8 shard samples per core
CB = C // 128                      # 4 channel blocks
NP = CB * B_SH                     # 32 planes per core
HW = H * W                         # 784
PR, PC = H + 2, W + 2              # padded plane 30 x 30
PLANE = PR * PC + PC               # 930: one dummy pad row per plane so
                                   # every shifted window slice stays
                                   # in-bounds (the extra row is never read)
HALF = H // 2                      # 14 rows per half-plane chunk
NCHUNK = HALF * W                  # 392 columns per conv psum chunk
NC9 = B_SH + 1                     # 9 pooled columns per cb (8 shard + track)
NCOL = CB * NC9                    # 36

# taps in row-major (dy, dx) order, center (0,0) excluded (folded into the
# combine seed)
TAPS8 = [(dy, dx) for dy in (-1, 0, 1) for dx in (-1, 0, 1)
         if not (dy == 0 and dx == 0)]

# plane -> engine: 'P' = PE (seed+matmul), 'V' = DVE, 'G' = GPSIMD.
# plane index p = b*4 + cb.  GPSIMD has no 3-operand ops (and no PSUM
# access), so its taps cost two tensor_tensor passes at 0.42 efficiency --
# it can only carry 2 planes.
_PAT = {7: "PPGV"}
OWNER = [(_PAT.get(b, "PPPV"))[cb] for b in range(B_SH) for cb in range(CB)]

# mid-layer PSUM eviction engines, round-robin (GPSIMD cannot access PSUM)
EVICT_CYCLE = ["A"]
# DVE/GPSIMD-plane mid-layer writeback via (otherwise idle) DMA
WRITEBACK_DMA = True

# cellps (single PSUM bank) column layout
Z1TI, Z1TH, Z2H, Z1B = 0, 1, 2, 3   # z1 pre-activations
G1 = 12                              # cell1 track gates, 12 cols
G2 = 24                              # cell2 gates, up to 96 cols


def build_program(num_layers: int = 4, iters: int = 1):
    nc = bacc.Bacc("TRN2", target_bir_lowering=False, debug=False,
                   num_devices=N_CORES)

    x_d = nc.dram_tensor("x", [B_SH, C, H, W], F32, kind="ExternalInput").ap()
    diag_d = nc.dram_tensor("diag", [CB * 8 * 128, 128], F32R,
                            kind="ExternalInput").ap()
    w8_d = nc.dram_tensor("w8", [128, CB * 8], F32, kind="ExternalInput").ap()
    wih1t_d = nc.dram_tensor("wih1t", [C, 32], F32, kind="ExternalInput").ap()
    whh1t_d = nc.dram_tensor("whh1t", [C, 32], F32, kind="ExternalInput").ap()
    w2cat_d = nc.dram_tensor("w2cat", [65, 3 * C], F32,
                             kind="ExternalInput").ap()
    b1_d = nc.dram_tensor("b1", [32, 2], F32, kind="ExternalInput").ap()
    ksw_d = nc.dram_tensor("ksw", [128, CB], F32, kind="ExternalInput").ap()
    w4p1_d = nc.dram_tensor("w4p1", [128, CB], F32, kind="ExternalInput").ap()
    p0i_d = nc.dram_tensor("p0init", [128, CB], F32, kind="ExternalInput").ap()
    eye_d = nc.dram_tensor("eye", [128, 128], F32, kind="ExternalInput").ap()
    y_d = nc.dram_tensor("y", [B_SH, C, H, W], F32R,
                          kind="ExternalOutput").ap()

    with tile.TileContext(nc) as tc:
        with (
            tc.tile_pool(name="persist", bufs=1) as pp,
            tc.tile_pool(name="stagep", bufs=2) as sp,
            tc.tile_pool(name="daccv", bufs=6) as dvp,
            tc.tile_pool(name="daccg", bufs=2) as dgp,
            tc.tile_pool(name="gtmpp", bufs=1) as dgt,
            tc.tile_pool(name="ostp", bufs=2) as ostp,
            tc.tile_pool(name="sdiagp", bufs=3) as sdp,
            tc.tile_pool(name="convps", bufs=7, space="PSUM") as cvp,
            tc.tile_pool(name="cellps", bufs=1, space="PSUM") as clp,
        ):
            xpad = pp.tile([128, NP * PLANE], F32R, tag="xpad")
            diag_sb = pp.tile([128, CB * 8 * 128], F32R, tag="diag")
            w8_sb = pp.tile([128, CB * 8], F32, tag="w8")
            wih1t_sb = pp.tile([128, CB * 32], F32, tag="wih1t")
            whh1t_sb = pp.tile([128, CB * 32], F32, tag="whh1t")
            w2cat_sb = pp.tile([65, 3 * C], F32, tag="w2cat")
            b1_sb = pp.tile([32, 2], F32, tag="b1")
            ksw_sb = pp.tile([128, CB], F32, tag="ksw")
            w4p1_sb = pp.tile([128, CB], F32, tag="w4p1")
            p0i_sb = pp.tile([128, CB], F32, tag="p0init")
            eye_sb = pp.tile([128, 128], F32, tag="eye")

            P = pp.tile([128, NCOL], F32, tag="pooled")
            s_l = [pp.tile([128, NCOL], F32, tag=f"s{l}", name=f"s{l}")
                   for l in range(num_layers)]
            ct0s = pp.tile([128, num_layers * CB], F32, tag="ct0s")
            z2hs = pp.tile([32, num_layers], F32, tag="z2hs")
            ht0 = pp.tile([128, CB], F32, tag="ht0")
            zcat = pp.tile([65, 16], F32, tag="zcat")
            sg1 = pp.tile([128, 12], F32, tag="sg1")
            tm1 = pp.tile([128, 12], F32, tag="tm1")
            sg2 = pp.tile([128, 128], F32, tag="sg2")
            tm2 = pp.tile([128, 64], F32, tag="tm2")

            cellps = clp.tile([128, G2 + 96], F32, tag="cellps")

            # ---- constants in once ----
            for cb in range(CB):
                nc.scalar.dma_start(
                    diag_sb[:, cb * 1024:(cb + 1) * 1024].rearrange(
                        "p (blk m) -> p blk m", m=128),
                    diag_d[cb * 1024:(cb + 1) * 1024, :].rearrange(
                        "(blk k) m -> k blk m", k=128))
            nc.scalar.dma_start(w8_sb[:, :], w8_d)
            nc.scalar.dma_start(
                wih1t_sb[:, :].rearrange("p (cb m) -> p cb m", m=32),
                wih1t_d.rearrange("(cb k) m -> k cb m", k=128))
            nc.scalar.dma_start(
                whh1t_sb[:, :].rearrange("p (cb m) -> p cb m", m=32),
                whh1t_d.rearrange("(cb k) m -> k cb m", k=128))
            nc.scalar.dma_start(w2cat_sb[:, :], w2cat_d)
            nc.scalar.dma_start(b1_sb[:, :], b1_d)
            nc.scalar.dma_start(ksw_sb[:, :], ksw_d)
            nc.scalar.dma_start(w4p1_sb[:, :], w4p1_d)
            nc.scalar.dma_start(p0i_sb[:, :], p0i_d)
            nc.scalar.dma_start(eye_sb[:, :], eye_d)
            # ones row for the augmented-bias matmuls; rest of zcat is
            # rewritten each use
            nc.vector.memset(zcat[64:65, :], 1.0)
            # zero the pad borders once (memset can't write f32r; broadcast-
            # copy a zeroed f32 tile into just the border rows/cols)
            z0 = sp.tile([128, PC], F32, tag="stage", name="z0")
            nc.vector.memset(z0[:, :], 0.0)
            zsrc = z0[:, 0:1].unsqueeze(-1).unsqueeze(-1)
            bv = xpad[:, 0:NP * PLANE].rearrange(
                "p (pl r w) -> p pl r w", r=PR + 1, w=PC)
            nc.vector.tensor_copy(
                bv[:, :, 0:PR:PR - 1, :],
                zsrc.broadcast_to([128, NP, 2, PC]))
            nc.vector.tensor_copy(
                bv[:, :, 1:PR - 1, 0:PC:PC - 1],
                zsrc.broadcast_to([128, NP, PR - 2, 2]))

            flat = xpad[:, :]

            def intr(pl, r0, nr):
                """interior window [128, nr, 28] of plane pl at row r0."""
                off = pl * PLANE + (r0 + 1) * PC + 1
                return flat[:, off:off + nr * PC].rearrange(
                    "p (r w) -> p r w", w=PC)[:, :, 0:W]

            def shifted(pl, r0, dy, dx):
                off = pl * PLANE + (r0 + 1 + dy) * PC + 1 + dx
                return flat[:, off:off + HALF * PC].rearrange(
                    "p (r w) -> p r w", w=PC)[:, :, 0:W]

            def shiftedF(pl, dy, dx):
                off = pl * PLANE + (1 + dy) * PC + 1 + dx
                return flat[:, off:off + H * PC].rearrange(
                    "p (r w) -> p r w", w=PC)[:, :, 0:W]

            def scol(l, p):
                b, cb = p // CB, p % CB
                c = cb * NC9 + b
                return s_l[l][:, c:c + 1]

            # ---------------- gate chain ----------------

            def cell2_epilogue(l, g2cols, n, dst_cols):
                """sigma/tanh + state combine for cell2 over n columns per
                cb.  g2cols: start col of the 12 j-blocks (each n wide) in
                cellps.  dst_cols(cb) -> s_l dst AP [128, n]."""
                sgi = sg2[:, 0:CB * n]
                sgf = sg2[:, 32:32 + CB * n]
                sgc = sg2[:, 64:64 + CB * n]
                nc.scalar.activation(sgi, cellps[:, g2cols:g2cols + CB * n],
                                     ACTF.Sigmoid)
                nc.scalar.activation(
                    sgf, cellps[:, g2cols + CB * n:g2cols + 2 * CB * n],
                    ACTF.Sigmoid)
                nc.scalar.activation(
                    sgc, cellps[:, g2cols + 2 * CB * n:g2cols + 3 * CB * n],
                    ACTF.Tanh)
                nc.vector.tensor_tensor(tm2[:, 0:CB * n], sgi, sgc, ALU.mult)
                for cb in range(CB):
                    nc.vector.scalar_tensor_tensor(
                        tm2[:, 32 + cb * n:32 + cb * n + n],
                        sgf[:, cb * n:cb * n + n],
                        ct0s[:, l * CB + cb:l * CB + cb + 1],
                        tm2[:, cb * n:cb * n + n], ALU.mult, ALU.add)
                nc.scalar.activation(sg2[:, 96:96 + CB * n],
                                     tm2[:, 32:32 + CB * n], ACTF.Sigmoid)
                for cb in range(CB):
                    # s' = sigmoid + (1 + w_center); also P *= (s + ksum)
                    dst = dst_cols(cb)
                    nc.vector.tensor_scalar(
                        dst, sg2[:, 96 + cb * n:96 + cb * n + n],
                        w4p1_sb[:, cb:cb + 1], None, ALU.add)
                    nc.vector.tensor_scalar(
                        tm2[:, cb * n:cb * n + n], dst,
                        ksw_sb[:, cb:cb + 1], None, ALU.add)
                pcols = [None] * CB
                for cb in range(CB):
                    pcols[cb] = dst_cols(cb, pooled=True)
                    nc.vector.tensor_tensor(
                        pcols[cb], pcols[cb], tm2[:, cb * n:cb * n + n],
                        ALU.mult)

            def gates12(rhs, out0, n):
                for j in range(12):
                    nc.tensor.matmul(
                        cellps[:, out0 + j * n:out0 + (j + 1) * n],
                        w2cat_sb[:, j * 128:(j + 1) * 128], rhs,
                        start=True, stop=True)

            def track_chain(l):
                """cell1 for global sample 0 + cell2 for the tracked column;
                stores ct0/z2h/s'track for layer l and advances P track."""
                # cell1 z1 pre-activations
                for cb in range(CB):
                    nc.tensor.matmul(
                        cellps[0:32, Z1TI:Z1TI + 1],
                        wih1t_sb[:, cb * 32:(cb + 1) * 32],
                        P[:, cb * NC9 + B_SH:cb * NC9 + B_SH + 1],
                        start=(cb == 0), stop=(cb == CB - 1))
                if l == 0:
                    nc.vector.memset(cellps[0:32, Z1TH:Z1TH + 1], 0.0)
                else:
                    for cb in range(CB):
                        nc.tensor.matmul(
                            cellps[0:32, Z1TH:Z1TH + 1],
                            whh1t_sb[:, cb * 32:(cb + 1) * 32],
                            ht0[:, cb:cb + 1],
                            start=(cb == 0), stop=(cb == CB - 1))
                nc.scalar.activation(zcat[0:32, 0:1], cellps[0:32, 0:1],
                                     ACTF.Relu, bias=b1_sb[:, 0:1])
                nc.scalar.activation(zcat[32:64, 0:1], cellps[0:32, 1:2],
                                     ACTF.Relu, bias=b1_sb[:, 1:2])
                gates12(zcat[0:65, 0:1], G1, 1)
                nc.scalar.activation(sg1[:, 0:4], cellps[:, G1:G1 + 4],
                                     ACTF.Sigmoid)
                nc.scalar.activation(sg1[:, 4:8], cellps[:, G1 + 4:G1 + 8],
                                     ACTF.Sigmoid)
                nc.scalar.activation(sg1[:, 8:12], cellps[:, G1 + 8:G1 + 12],
                                     ACTF.Tanh)
                ct_new = ct0s[:, l * CB:(l + 1) * CB]
                nc.vector.tensor_tensor(tm1[:, 0:4], sg1[:, 0:4],
                                        sg1[:, 8:12], ALU.mult)
                if l == 0:
                    nc.vector.tensor_copy(ct_new, tm1[:, 0:4])
                else:
                    nc.vector.tensor_tensor(
                        tm1[:, 4:8], sg1[:, 4:8],
                        ct0s[:, (l - 1) * CB:l * CB], ALU.mult)
                    nc.vector.tensor_tensor(ct_new, tm1[:, 0:4],
                                            tm1[:, 4:8], ALU.add)
                nc.scalar.activation(ht0[:, :], ct_new, ACTF.Sigmoid)
                # cell2 hh path from the updated state
                for cb in range(CB):
                    nc.tensor.matmul(
                        cellps[0:32, Z2H:Z2H + 1],
                        whh1t_sb[:, cb * 32:(cb + 1) * 32],
                        ht0[:, cb:cb + 1],
                        start=(cb == 0), stop=(cb == CB - 1))
                nc.scalar.activation(z2hs[:, l:l + 1], cellps[0:32, 2:3],
                                     ACTF.Relu, bias=b1_sb[:, 1:2])
                # cell2 for the tracked column
                nc.vector.tensor_copy(zcat[0:32, 10:11], zcat[0:32, 0:1])
                nc.vector.tensor_copy(zcat[32:64, 10:11], z2hs[:, l:l + 1])
                gates12(zcat[0:65, 10:11], G2, 1)
                cell2_epilogue(
                    l, G2, 1,
                    lambda cb, pooled=False:
                        (P if pooled else s_l[l])[
                            :, cb * NC9 + B_SH:cb * NC9 + B_SH + 1])

            def sample_s_all(b):
                """s for ALL layers of one shard sample, as soon as its
                pooled sums land (pooled evolves closed-form, so no layer
                needs any conv result)."""
                for l in range(num_layers):
                    for cb in range(CB):
                        nc.tensor.matmul(
                            cellps[0:32, Z1B:Z1B + 1],
                            wih1t_sb[:, cb * 32:(cb + 1) * 32],
                            P[:, cb * NC9 + b:cb * NC9 + b + 1],
                            start=(cb == 0), stop=(cb == CB - 1))
                    nc.scalar.activation(zcat[0:32, 1:2],
                                         cellps[0:32, Z1B:Z1B + 1],
                                         ACTF.Relu, bias=b1_sb[:, 0:1])
                    nc.vector.tensor_copy(zcat[32:64, 1:2], z2hs[:, l:l + 1])
                    gates12(zcat[0:65, 1:2], G2, 1)
                    cell2_epilogue(
                        l, G2, 1,
                        lambda cb, pooled=False:
                            (P if pooled else s_l[l])[
                                :, cb * NC9 + b:cb * NC9 + b + 1])

            # ---------------- conv planes ----------------

            evict_ctr = [0]

            def evict_chunk(dst, ps_flat, shaped_dst):
                """copy a finished PSUM chunk out via ACT/DVE/GPSIMD."""
                eng = EVICT_CYCLE[evict_ctr[0] % len(EVICT_CYCLE)]
                evict_ctr[0] += 1
                if eng == "A":
                    nc.scalar.activation(dst, ps_flat if not shaped_dst else
                                         ps_flat.rearrange(
                                             "p (r w) -> p r w", w=W),
                                         ACTF.Copy)
                elif eng == "V":
                    nc.vector.tensor_copy(dst, ps_flat if not shaped_dst else
                                          ps_flat.rearrange(
                                              "p (r w) -> p r w", w=W))
                else:
                    nc.gpsimd.tensor_copy(dst, ps_flat if not shaped_dst else
                                          ps_flat.rearrange(
                                              "p (r w) -> p r w", w=W))

            def pe_plane_taps(l, p):
                b, cb = p // CB, p % CB
                # center tap + combine folded into a per-plane diagonal
                # diag(w_center + s) built on the ACT engine; the whole
                # 9-matmul accumulation group then stays on the PE
                sdiag = sdp.tile([128, 128], F32R, tag="sdiag", name="sdiag")
                nc.scalar.activation(sdiag[:, :], eye_sb[:, :], ACTF.Copy,
                                     scale=scol(l, p))
                chunks = []
                for hf in range(2):
                    r0 = hf * HALF
                    ps = cvp.tile([128, NCHUNK], F32, tag="cps", name="cps")
                    nc.tensor.matmul(ps[:, :], sdiag[:, :],
                                     intr(p, r0, HALF),
                                     start=True, stop=False)
                    for ti, (dy, dx) in enumerate(TAPS8):
                        nc.tensor.matmul(
                            ps[:, :],
                            diag_sb[:, (cb * 8 + ti) * 128:
                                    (cb * 8 + ti + 1) * 128],
                            shifted(p, r0, dy, dx),
                            start=False, stop=(ti == 7))
                    chunks.append(ps)
                return chunks

            def pe_plane_evict(l, p, chunks, last):
                b, cb = p // CB, p % CB
                if last:
                    ost = ostp.tile([128, HW], F32R, tag="ost", name="ost")
                    for hf in range(2):
                        evict_chunk(ost[:, hf * NCHUNK:(hf + 1) * NCHUNK],
                                    chunks[hf][:, :], shaped_dst=False)
                    nc.scalar.dma_start(
                        y_d[b, cb * 128:(cb + 1) * 128, :, :],
                        ost[:, :].rearrange("p (h w) -> p h w", w=W))
                else:
                    for hf in range(2):
                        evict_chunk(intr(p, hf * HALF, HALF),
                                    chunks[hf][:, :], shaped_dst=True)

            xv = flat[:, 0:NP * PLANE].rearrange("p (pl z) -> p pl z",
                                                 z=PLANE)

            def pair_view(p0, dy, dx, dp=CB):
                """[128, 2, 28, 28] window over planes p0 and p0+dp."""
                woff = (1 + dy) * PC + 1 + dx
                return xv[:, p0:p0 + dp + 1:dp,
                          woff:woff + H * PC].rearrange(
                    "p q (r w) -> p q r w", w=PC)[:, :, :, 0:W]

            def v_pair(l, b0, last):
                """two same-cb DVE planes (samples b0, b0+1); STT APs are
                capped at 3 dims so taps stay per-plane."""
                cb = CB - 1
                for k in range(2):
                    p = b0 * CB + k * CB + cb
                    acc = dvp.tile([128, HW], F32R, tag="dacc", name="dacc")
                    av = acc[:, :].rearrange("p (h w) -> p h w", w=W)
                    nc.vector.tensor_scalar(av, intr(p, 0, H), scol(l, p),
                                            None, ALU.mult)
                    for ti, (dy, dx) in enumerate(TAPS8):
                        nc.vector.scalar_tensor_tensor(
                            av, shiftedF(p, dy, dx),
                            w8_sb[:, cb * 8 + ti:cb * 8 + ti + 1], av,
                            ALU.mult, ALU.add)
                    b = p // CB
                    if last:
                        nc.sync.dma_start(
                            y_d[b, cb * 128:(cb + 1) * 128, :, :], av)
                    else:
                        nc.sync.dma_start(intr(p, 0, H), av)

            def vg_plane(l, p, last):
                b, cb = p // CB, p % CB
                if OWNER[p] == "V":
                    raise AssertionError("V planes go through v_pair")
                else:
                    # GPSIMD: no TensorScalarPtr -- weighted taps as two
                    # tensor_tensor passes with the weight broadcast
                    acc = dgp.tile([128, HW], F32R, tag="dacc", name="dacc")
                    av = acc[:, :].rearrange("p (h w) -> p h w", w=W)
                    tmp = dgt.tile([128, HW], F32, tag="gtmp", name="gtmp")
                    tv = tmp[:, :].rearrange("p (h w) -> p h w", w=W)
                    bc = lambda col: col.unsqueeze(-1).broadcast_to(
                        [128, H, W])
                    # keep the whole GPSIMD plane self-contained (center via
                    # tensor_tensor, writeback via SWDGE) so the slow Pool
                    # never back-pressures the ACT or sync-DMA queues
                    nc.gpsimd.tensor_tensor(av, intr(p, 0, H),
                                            bc(scol(l, p)), ALU.mult)
                    for ti, (dy, dx) in enumerate(TAPS8):
                        nc.gpsimd.tensor_tensor(
                            tv, shiftedF(p, dy, dx),
                            bc(w8_sb[:, cb * 8 + ti:cb * 8 + ti + 1]),
                            ALU.mult)
                        nc.gpsimd.tensor_tensor(av, av, tv, ALU.add)
                dma = nc.sync if OWNER[p] == "V" else nc.gpsimd
                if last:
                    dma.dma_start(y_d[b, cb * 128:(cb + 1) * 128, :, :], av)
                elif WRITEBACK_DMA:
                    dma.dma_start(intr(p, 0, H), av)
                else:
                    nc.vector.tensor_copy(intr(p, 0, H), av)

            # ---------------- staging + emission ----------------

            def stage_sample(b):
                stage = sp.tile([128, CB * HW], F32, tag="stage", name="stage")
                nc.sync.dma_start(
                    stage[:, :].rearrange("p (cb hw) -> p cb hw", hw=HW),
                    x_d[b, :, :, :].rearrange("(cb k) h w -> k cb (h w)",
                                              k=128))
                for cb in range(CB):
                    p = b * CB + cb
                    seg = stage[:, cb * HW:(cb + 1) * HW].rearrange(
                        "p (h w) -> p h w", w=W)
                    pcol = P[:, cb * NC9 + b:cb * NC9 + b + 1]
                    if OWNER[p] == "P":
                        nc.scalar.activation(intr(p, 0, H), seg, ACTF.Copy,
                                             accum_out=pcol)
                    else:
                        nc.vector.tensor_scalar(intr(p, 0, H), seg, 1.0, 0.0,
                                                ALU.mult, ALU.add,
                                                accum_out=pcol)

            def emit_body():
                # tracked pooled seed, then the whole track chain (depends
                # only on host-precomputed p0init -> runs during staging)
                with tc.high_priority():
                    nc.vector.tensor_copy(P[:, B_SH::NC9], p0i_sb[:, :])
                    for l in range(num_layers):
                        track_chain(l)

                # Layer 0 overlapped with staging; the whole gate chain
                # for ALL layers is emitted before any bulk DVE/Pool conv
                # work so s never queues behind it (no layer barriers).
                # Keep at most 2 PE planes (4 PSUM chunks) pending
                # eviction: + 2 chunks being diag-seeded stays within the
                # 7-bank conv pool, and the eviction op trails the matmuls
                # far enough that ACT/DVE never block on the PE.
                pend = []

                def flush_pend(n=0, plane=None):
                    while pend and (len(pend) > n or any(
                            q == plane for q, _l, _c in pend)):
                        q, ql, ch = pend.pop(0)
                        pe_plane_evict(ql, q, ch, ql == num_layers - 1)

                def pe_planes_of(l, b):
                    for cb in range(CB):
                        p = b * CB + cb
                        if OWNER[p] == "P":
                            flush_pend(2, p)
                            pend.append((p, l, pe_plane_taps(l, p)))

                # layer-major PE order: a plane's eviction lands a whole
                # layer before its next-layer taps re-read it, so the PE
                # never waits on evictions
                for b in range(B_SH):
                    with tc.high_priority():
                        stage_sample(b)
                        sample_s_all(b)
                    if b > 0:
                        pe_planes_of(0, b - 1)
                pe_planes_of(0, B_SH - 1)
                for l in range(1, num_layers):
                    for b in range(B_SH):
                        pe_planes_of(l, b)
                flush_pend(0)

                # DVE/GPSIMD planes layer-major: writeback latency hides
                # across the other planes of the same layer
                for l in range(num_layers):
                    last = l == num_layers - 1
                    for b0 in range(0, B_SH, 2):
                        v_pair(l, b0, last)
                    for b in range(B_SH):
                        for cb in range(CB):
                            p = b * CB + cb
                            if OWNER[p] == "G":
                                vg_plane(l, p, last)

            if iters == 1:
                emit_body()
            else:
                with tc.For_i(0, iters, 1):
                    emit_body()

    nc.compile()
    return nc


def prep_inputs(x, w_ih_l1, b_ih_l1, w_ih_l2, b_ih_l2,
                w_hh_l1, b_hh_l1, w_hh_l2, b_hh_l2, dw_kernel):
    """Host-side prep: per-core input maps (weights replicated)."""
    x = np.ascontiguousarray(np.asarray(x, dtype=np.float32))
    dw = np.asarray(dw_kernel, np.float32).reshape(C, 9)
    taps8_t = [dy * 3 + dx + 4 for (dy, dx) in TAPS8]
    diag = np.zeros((CB, 8, 128, 128), np.float32)
    w8 = np.zeros((128, CB * 8), np.float32)
    idx = np.arange(128)
    for cb in range(CB):
        for ti, t in enumerate(taps8_t):
            diag[cb, ti, idx, idx] = dw[cb * 128:(cb + 1) * 128, t]
            w8[:, cb * 8 + ti] = dw[cb * 128:(cb + 1) * 128, t]
    w4 = dw[:, 4].reshape(CB, 128).T                      # [128, CB]
    ksum = dw.sum(axis=1).reshape(CB, 128).T
    w2cat = np.concatenate(
        [np.asarray(w_ih_l2, np.float32).T,
         np.asarray(w_hh_l2, np.float32).T,
         (np.asarray(b_ih_l2, np.float32)
          + np.asarray(b_hh_l2, np.float32))[None, :]], axis=0)  # [65, 3C]
    common = {
        "diag": diag.reshape(CB * 8 * 128, 128),
        "w8": w8,
        "wih1t": np.ascontiguousarray(
            (np.asarray(w_ih_l1, np.float32) / HW).T),
        "whh1t": np.ascontiguousarray(np.asarray(w_hh_l1, np.float32).T),
        "w2cat": np.ascontiguousarray(w2cat),
        "b1": np.ascontiguousarray(np.stack(
            [np.asarray(b_ih_l1, np.float32),
             np.asarray(b_hh_l1, np.float32)], axis=1)),
        "ksw": np.ascontiguousarray(ksum - w4),           # s + ksum = s' + ksw
        "w4p1": np.ascontiguousarray(1.0 + w4),           # s' = sig + w4p1
        "p0init": np.ascontiguousarray(
            x[0].reshape(C, HW).sum(axis=1).reshape(CB, 128).T),
        "eye": np.eye(128, dtype=np.float32),
    }
    return [dict(common, x=np.ascontiguousarray(x[i * B_SH:(i + 1) * B_SH]))
            for i in range(N_CORES)]


_cache = {}


def kernel(**inputs) -> np.ndarray:
    num_layers = int(inputs["num_layers"])
    if num_layers == 0:
        return np.asarray(inputs["x"], np.float32).copy()
    if num_layers not in _cache:
        _cache[num_layers] = build_program(num_layers=num_layers, iters=1)
    nc = _cache[num_layers]
    in_maps = prep_inputs(
        inputs["x"], inputs["w_ih_l1"], inputs["b_ih_l1"], inputs["w_ih_l2"],
        inputs["b_ih_l2"], inputs["w_hh_l1"], inputs["b_hh_l1"],
        inputs["w_hh_l2"], inputs["b_hh_l2"], inputs["dw_kernel"])
    res = run_bass_kernel_spmd(nc, in_maps, list(range(N_CORES)))
    return np.concatenate([res.results[i]["y"] for i in range(N_CORES)],
                          axis=0).astype(np.float32)


# revision 30
# speedup vs baseline: 1.5141x; 1.5141x over previous
"""Trainium2 Bass kernel for the recurrent-SE / depthwise-conv attention block.

Math per layer (faithful to the reference):
    pooled = mean(x, (2,3))                      # [B, C]
    ht, ct = cell(pooled, ht, ct)                # DSU cell, state [B, C]
    out_h, _ = cell(pooled, ht[0], ct[0])        # GLOBAL batch-0 state bcast
    x = x * (1 + out_h)[:, :, None, None] + dwconv3x3(x)

Sharding: data-parallel over batch, 8 samples/core.  The global sample-0
recurrent state that cell2 broadcasts is NOT carried as replica planes;
instead pooled(x_0) evolves by the closed recurrence
    pooled_0' = pooled_0 * (s_0 + sum_t w_t)
which is exact up to SAME-padding border terms (measured rel err ~3e-5 on
the final output).  Each core seeds it by reducing sample-0's planes once.

Per core:
  - x lives in SBUF in a zero-padded [30x30] per-(channel-block, sample)
    plane layout, channels on partitions (4 blocks of 128 channels), f32r.
  - dwconv3x3 runs on the TensorEngine as 9 accumulating matmuls per
    half-plane chunk with host-prebuilt diagonal tap matrices.
  - A few planes per layer are instead convolved on the (otherwise
    underused) DVE as 9 shifted multiply-accumulates, sized so PE and DVE
    finish a layer together.
  - The combine x*s + conv is one DVE scalar_tensor_tensor per half-plane
    reading the conv result straight from PSUM; its accum_out yields the
    pooled sums for the next layer (1/784 folded into w_ih_l1 host-side).
    The first SPILL planes of each layer go through an ACT spill instead,
    the evicts woven between the cell chain's own ACT ops, so PSUM banks
    recycle at PE pace while the serial chain computes s.
  - Input staging: DMA contiguous planes to a stage buffer, then one DVE
    tensor_scalar per plane into the padded layout (f32->f32r rounding)
    whose accum_out is the layer-0 pooled sum; layer-0 convs for the first
    planes are interleaved with the input groups so the PE works (and its
    HAM clock-gate stays warm) while input streams in.
"""

import numpy as np

import concourse.bacc as bacc
import concourse.bass as bass
import concourse.mybir as mybir
import concourse.tile as tile
from concourse.bass_utils import run_bass_kernel_spmd

F32 = mybir.dt.float32
F32R = mybir.dt.float32r
ALU = mybir.AluOpType
ACTF = mybir.ActivationFunctionType
AX = mybir.AxisListType

N_CORES = 8
B_FULL, C, H, W = 64, 512, 28, 28
B_SH = B_FULL // N_CORES           # 8 shard samples per core
CB = C // 128                      # 4 channel blocks
NP = CB * B_SH                     # 32 planes per core
NCOL = CB * (B_SH + 1)             # 36 cell columns (8 shard + 1 tracked)/cb
HW = H * W                         # 784
PR, PC = H + 2, W + 2              # padded plane 30 x 30
PLANE = PR * PC                    # 900
HALF = H // 2                      # 14 rows per half-plane chunk
NCHUNK = HALF * W                  # 392 columns per conv matmul
G3 = 3 * NCOL                      # 108 gate columns
NC9 = B_SH + 1                     # 9 cell columns per cb

# packed single-bank cell PSUM layout (columns of cellps)
ZC1 = 2 * NC9 + 1                  # z1 pre-activations [33p, 19]
GI0, GI1 = ZC1, ZC1 + G3           # g_i (+bias) 12 x 9
GH0, GH1 = GI1, GI1 + G3           # g_h 12 x 9
G20, G21 = GH1, GH1 + 12           # g_h2 (batch-0 bcast) 12 x 1

SPILL = 8                          # planes per layer evicted via ACT
K_DVE = 5                          # planes per layer convolved on the DVE
NPE = NP - K_DVE                   # planes convolved on the PE (0..NPE-1)


def ccol(pl):
    """cell/gate column for plane pl (shard cols 0..7, tracked col 8)."""
    return (pl // B_SH) * NC9 + pl % B_SH


def build_program(num_layers: int = 4, iters: int = 1):
    nc = bacc.Bacc("TRN2", target_bir_lowering=False, debug=False,
                   num_devices=N_CORES)

    x_d = nc.dram_tensor("x", [B_SH, C, H, W], F32, kind="ExternalInput").ap()
    diag_d = nc.dram_tensor("diag", [CB * 9 * 128, 128], F32R,
                            kind="ExternalInput").ap()
    wih1t_d = nc.dram_tensor("wih1t", [C, 32], F32, kind="ExternalInput").ap()
    whh1t_d = nc.dram_tensor("whh1t", [C, 32], F32, kind="ExternalInput").ap()
    wih2t_d = nc.dram_tensor("wih2t", [33, 3 * C], F32, kind="ExternalInput").ap()
    whh2t_d = nc.dram_tensor("whh2t", [33, 3 * C], F32, kind="ExternalInput").ap()
    b1_d = nc.dram_tensor("b1", [32, 2], F32, kind="ExternalInput").ap()
    ksum_d = nc.dram_tensor("ksum", [128, CB], F32, kind="ExternalInput").ap()
    dwv_d = nc.dram_tensor("dwv", [128, CB * 9], F32, kind="ExternalInput").ap()
    p0i_d = nc.dram_tensor("p0init", [128, CB], F32, kind="ExternalInput").ap()
    y_d = nc.dram_tensor("y", [B_SH, C, H, W], F32, kind="ExternalOutput").ap()

    with tile.TileContext(nc) as tc:
        with (
            tc.tile_pool(name="persist", bufs=1) as pp,
            tc.tile_pool(name="stagep", bufs=3) as sp,
            tc.tile_pool(name="spillp", bufs=2 * SPILL - 1) as spl,
            tc.tile_pool(name="convps", bufs=7, space="PSUM") as cvp,
            tc.tile_pool(name="cellps", bufs=1, space="PSUM") as clp,
        ):
            # +PC slack so the last plane's shifted window slice stays in range
            xpad = pp.tile([128, NP * PLANE + PC], F32R, tag="xpad")
            diag_sb = pp.tile([128, CB * 9 * 128], F32R, tag="diag")
            wih1t_sb = pp.tile([128, CB * 32], F32, tag="wih1t")
            whh1t_sb = pp.tile([128, CB * 32], F32, tag="whh1t")
            wih2t_sb = pp.tile([33, 3 * C], F32, tag="wih2t")
            whh2t_sb = pp.tile([33, 3 * C], F32, tag="whh2t")
            b1_sb = pp.tile([32, 2], F32, tag="b1")
            ksum_sb = pp.tile([128, CB], F32, tag="ksum")
            dwv_sb = pp.tile([128, CB * 9], F32, tag="dwv")
            p0i_sb = pp.tile([128, CB], F32, tag="p0init")

            pooled = pp.tile([128, NCOL], F32, tag="pooled")
            poolacc = pp.tile([128, NP * 2], F32, tag="poolacc")
            ht = pp.tile([128, NCOL], F32, tag="ht")
            ct = pp.tile([128, NCOL], F32, tag="ct")
            z1 = pp.tile([33, ZC1], F32, tag="z1")
            gates = pp.tile([128, G3], F32, tag="gates")
            sgi = pp.tile([128, G3], F32, tag="sgi")
            tmt = pp.tile([128, G3], F32, tag="tmt")
            gi_sb = pp.tile([128, G3], F32, tag="gi_sb")
            s_sb = pp.tile([128, NCOL], F32, tag="s_sb")
            s0k = pp.tile([128, CB], F32, tag="s0k")

            cellps = clp.tile([128, G21], F32, tag="cellps")

            # constants in once
            for cb in range(CB):
                # per-cb chunks so plane-0 convs wait on 1/4 of the weights
                nc.scalar.dma_start(
                    diag_sb[:, cb * 1152:(cb + 1) * 1152].rearrange(
                        "p (blk m) -> p blk m", m=128),
                    diag_d[cb * 1152:(cb + 1) * 1152, :].rearrange(
                        "(blk k) m -> k blk m", k=128))
            nc.scalar.dma_start(
                wih1t_sb[:, :].rearrange("p (cb m) -> p cb m", m=32),
                wih1t_d.rearrange("(cb k) m -> k cb m", k=128))
            nc.scalar.dma_start(
                whh1t_sb[:, :].rearrange("p (cb m) -> p cb m", m=32),
                whh1t_d.rearrange("(cb k) m -> k cb m", k=128))
            nc.scalar.dma_start(wih2t_sb[:, :], wih2t_d)
            nc.scalar.dma_start(whh2t_sb[:, :], whh2t_d)
            nc.scalar.dma_start(b1_sb[:, :], b1_d)
            nc.scalar.dma_start(ksum_sb[:, :], ksum_d)
            nc.scalar.dma_start(dwv_sb[:, :], dwv_d)
            nc.scalar.dma_start(p0i_sb[:, :], p0i_d)
            # ones row for the augmented-bias matmuls
            nc.vector.memset(z1[32:33, :], 1.0)
            # zero the pad borders once; interiors are overwritten each
            # layer, borders stay zero forever. (memset can't write f32r --
            # the fp32r matmuls need their input rounded by a converting
            # engine op -- so zero a small f32 tile and broadcast-copy it
            # into just the border rows/cols, not the whole planes)
            z0 = sp.tile([128, PC], F32, tag="stage", name="z0")
            nc.vector.memset(z0[:, :], 0.0)
            zsrc = z0[:, 0:1].unsqueeze(-1).unsqueeze(-1)
            bv = xpad[:, 0:NP * PLANE].rearrange(
                "p (pl r w) -> p pl r w", r=PR, w=PC)
            nc.vector.tensor_copy(
                bv[:, :, 0:PR:PR - 1, :],
                zsrc.broadcast_to([128, NP, 2, PC]))
            nc.vector.tensor_copy(
                bv[:, :, 1:PR - 1, 0:PC:PC - 1],
                zsrc.broadcast_to([128, NP, PR - 2, 2]))
            nc.vector.tensor_copy(
                xpad[:, NP * PLANE:NP * PLANE + PC], z0[:, 0:PC])

            flat = xpad[:, :]

            def intr(pl, r0, nr):
                """interior window [128, nr, 28] of plane pl at row r0."""
                off = pl * PLANE + (r0 + 1) * PC + 1
                return flat[:, off:off + nr * PC].rearrange(
                    "p (r w) -> p r w", w=PC)[:, :, 0:W]

            def shifted(pl, r0, dy, dx):
                off = pl * PLANE + (r0 + 1 + dy) * PC + 1 + dx
                return flat[:, off:off + HALF * PC].rearrange(
                    "p (r w) -> p r w", w=PC)[:, :, 0:W]

            def cell_stage(st, first_layer):
                """The DSU cell in 5 stages so PE work can interleave with
                conv planes.  pooled, ht, ct -> new ht, ct; s = 1+out_h."""
                if st == 0:  # z1 pre-activations (PE)
                    if not first_layer:
                        # layer 0's ih matmuls are emitted inside emit_input
                        for cb in range(CB):
                            nc.tensor.matmul(
                                cellps[0:32, 0:NC9],
                                wih1t_sb[:, cb * 32:(cb + 1) * 32],
                                pooled[:, cb * NC9:(cb + 1) * NC9],
                                start=(cb == 0), stop=(cb == CB - 1))
                    if first_layer:
                        # ht == 0 -> hh path contributes relu(b_hh1)
                        nc.vector.memset(cellps[0:32, NC9:2 * NC9], 0.0)
                    else:
                        for cb in range(CB):
                            nc.tensor.matmul(
                                cellps[0:32, NC9:2 * NC9],
                                whh1t_sb[:, cb * 32:(cb + 1) * 32],
                                ht[:, cb * NC9:(cb + 1) * NC9],
                                start=(cb == 0), stop=(cb == CB - 1))
                elif st == 1:  # relu, then gate matmuls (PE bulk)
                    nc.scalar.activation(z1[0:32, 0:NC9], cellps[0:32, 0:NC9],
                                         ACTF.Relu, bias=b1_sb[:, 0:1])
                    nc.scalar.activation(z1[0:32, NC9:2 * NC9],
                                         cellps[0:32, NC9:2 * NC9],
                                         ACTF.Relu, bias=b1_sb[:, 1:2])
                    for g in range(3):
                        for cb in range(CB):
                            co = (g * CB + cb) * NC9
                            wsl = slice(g * C + cb * 128,
                                        g * C + (cb + 1) * 128)
                            nc.tensor.matmul(
                                cellps[:, GI0 + co:GI0 + co + NC9],
                                wih2t_sb[:, wsl], z1[:, 0:NC9],
                                start=True, stop=True)
                            nc.tensor.matmul(
                                cellps[:, GH0 + co:GH0 + co + NC9],
                                whh2t_sb[:, wsl], z1[:, NC9:2 * NC9],
                                start=True, stop=True)
                elif st == 2:  # cell 1 state update (DVE/ACT)
                    nc.vector.tensor_copy(gi_sb[:, :], cellps[:, GI0:GI1])
                    nc.vector.tensor_tensor(gates[:, :], gi_sb[:, :],
                                            cellps[:, GH0:GH1], ALU.add)
                    nc.scalar.activation(sgi[:, 0:NCOL], gates[:, 0:NCOL],
                                         ACTF.Sigmoid)
                    nc.scalar.activation(sgi[:, NCOL:2 * NCOL],
                                         gates[:, NCOL:2 * NCOL],
                                         ACTF.Sigmoid)
                    nc.scalar.activation(sgi[:, 2 * NCOL:G3],
                                         gates[:, 2 * NCOL:G3], ACTF.Tanh)
                    nc.vector.tensor_tensor(tmt[:, 0:NCOL], sgi[:, 0:NCOL],
                                            sgi[:, 2 * NCOL:G3], ALU.mult)
                    if first_layer:
                        nc.vector.tensor_copy(ct[:, :], tmt[:, 0:NCOL])
                    else:
                        nc.vector.tensor_tensor(
                            tmt[:, NCOL:2 * NCOL],
                            sgi[:, NCOL:2 * NCOL], ct[:, :], ALU.mult)
                        nc.vector.tensor_tensor(ct[:, :], tmt[:, 0:NCOL],
                                                tmt[:, NCOL:2 * NCOL],
                                                ALU.add)
                    nc.scalar.activation(ht[:, :], ct[:, :], ACTF.Sigmoid)
                elif st == 3:  # cell 2 hh path from sample-0 state (PE)
                    for cb in range(CB):
                        c0 = cb * NC9 + B_SH
                        nc.tensor.matmul(
                            cellps[0:32, 2 * NC9:2 * NC9 + 1],
                            whh1t_sb[:, cb * 32:(cb + 1) * 32],
                            ht[:, c0:c0 + 1],
                            start=(cb == 0), stop=(cb == CB - 1))
                    nc.scalar.activation(z1[0:32, 2 * NC9:2 * NC9 + 1],
                                         cellps[0:32, 2 * NC9:2 * NC9 + 1],
                                         ACTF.Relu, bias=b1_sb[:, 1:2])
                    for g in range(3):
                        for cb in range(CB):
                            j = G20 + g * CB + cb
                            nc.tensor.matmul(
                                cellps[:, j:j + 1],
                                whh2t_sb[:, g * C + cb * 128:
                                         g * C + (cb + 1) * 128],
                                z1[:, 2 * NC9:2 * NC9 + 1],
                                start=True, stop=True)
                else:  # st == 4: cell 2 -> s = 1 + out_h (DVE/ACT)
                    # gates2 = (g_i + bias) + g_h2 broadcast over batch
                    nc.vector.tensor_tensor(
                        gates[:, :].rearrange("p (j b) -> p j b", b=NC9),
                        gi_sb[:, :].rearrange("p (j b) -> p j b", b=NC9),
                        cellps[:, G20:G21].unsqueeze(-1).broadcast_to(
                            [128, 12, NC9]),
                        ALU.add)
                    nc.scalar.activation(sgi[:, 0:NCOL], gates[:, 0:NCOL],
                                         ACTF.Sigmoid)
                    nc.scalar.activation(sgi[:, NCOL:2 * NCOL],
                                         gates[:, NCOL:2 * NCOL],
                                         ACTF.Sigmoid)
                    nc.scalar.activation(sgi[:, 2 * NCOL:G3],
                                         gates[:, 2 * NCOL:G3], ACTF.Tanh)
                    nc.vector.tensor_tensor(tmt[:, 0:NCOL], sgi[:, 0:NCOL],
                                            sgi[:, 2 * NCOL:G3], ALU.mult)
                    # ncx2 = sig(f2)*ct[0] + sig(i2)*tanh(c2)
                    for cb in range(CB):
                        bs = cb * NC9
                        nc.vector.scalar_tensor_tensor(
                            tmt[:, NCOL + bs:NCOL + bs + NC9],
                            sgi[:, NCOL + bs:NCOL + bs + NC9],
                            ct[:, bs + B_SH:bs + B_SH + 1],
                            tmt[:, bs:bs + NC9],
                            ALU.mult, ALU.add)
                    nc.scalar.activation(tmt[:, 2 * NCOL:G3],
                                         tmt[:, NCOL:2 * NCOL],
                                         ACTF.Sigmoid)
                    nc.vector.tensor_scalar(s_sb[:, :], tmt[:, 2 * NCOL:G3],
                                            1.0, None, ALU.add)

            def shiftedF(pl, dy, dx):
                """full-plane shifted window [128, 28, 28]."""
                off = pl * PLANE + (1 + dy) * PC + 1 + dx
                return flat[:, off:off + H * PC].rearrange(
                    "p (r w) -> p r w", w=PC)[:, :, 0:W]

            def emit_conv(pl, spill):
                """18 conv matmuls of one plane.  spill=True routes the PSUM
                result through an ACT copy (not gated on s) so banks recycle
                at PE pace while the cell chain computes s."""
                cb = pl // B_SH
                chunks = []
                for hf in range(2):
                    r0 = hf * HALF
                    ps = cvp.tile([128, NCHUNK], F32, tag="cps", name="cps")
                    for t in range(9):
                        dy, dx = t // 3 - 1, t % 3 - 1
                        nc.tensor.matmul(
                            ps[:, :],
                            diag_sb[:, (cb * 9 + t) * 128:
                                    (cb * 9 + t + 1) * 128],
                            shifted(pl, r0, dy, dx),
                            start=(t == 0), stop=(t == 8))
                    if spill:
                        sc = spl.tile([128, NCHUNK], F32, tag="spill",
                                      name="sc")
                        nc.scalar.copy(sc[:, :], ps[:, :])
                        chunks.append(sc)
                    else:
                        chunks.append(ps)
                return chunks

            def emit_combine(pl, chunks, last):
                cb, b = pl // B_SH, pl % B_SH
                col = cb * NC9 + b
                ost = (sp.tile([128, HW], F32, tag="ost", name="ost")
                       if last else None)
                for hf in range(2):
                    r0 = hf * HALF
                    src = intr(pl, r0, HALF)
                    if last:
                        dst = ost[:, r0 * W:(r0 + HALF) * W].rearrange(
                            "p (r w) -> p r w", w=W)
                        acc = None
                    else:
                        dst = src
                        acc = poolacc[:, pl * 2 + hf:pl * 2 + hf + 1]
                    nc.vector.scalar_tensor_tensor(
                        dst, src, s_sb[:, col:col + 1],
                        chunks[hf][:, :].rearrange("p (r w) -> p r w", w=W),
                        ALU.mult, ALU.add, accum_out=acc)
                if last:
                    eng = nc.sync if pl % 2 == 0 else nc.scalar
                    eng.dma_start(
                        y_d[b, cb * 128:(cb + 1) * 128, :, :],
                        ost[:, :].rearrange("p (h w) -> p h w", w=W))

            def dve_tap_ops(pl):
                """Generator of the 9 DVE conv-tap ops for one plane (the
                conv of a DVE-offloaded plane); yields after each emission
                so the caller can interleave them between combines."""
                cb = pl // B_SH
                acc = sp.tile([128, HW], F32, tag="dacc", name="dacc")
                av = acc[:, :].rearrange("p (r w) -> p r w", w=W)
                for t in range(9):
                    dy, dx = t // 3 - 1, t % 3 - 1
                    wap = dwv_sb[:, cb * 9 + t:cb * 9 + t + 1]
                    if t == 0:
                        nc.vector.tensor_scalar(
                            av, shiftedF(pl, dy, dx), wap, None, ALU.mult)
                    else:
                        nc.vector.scalar_tensor_tensor(
                            av, shiftedF(pl, dy, dx), wap, av,
                            ALU.mult, ALU.add)
                    yield None
                chunks = [acc[:, 0:HALF * W], acc[:, HALF * W:HW]]
                emit_combine(pl, chunks, pl_last_flag[0])

            def input_group(cb, gi, b0, nb):
                stage = sp.tile([128, 2 * HW], F32, tag="stage", name="stage")
                eng = nc.sync if gi % 2 == 0 else nc.scalar
                eng.dma_start(
                    stage[:, 0:nb * HW].rearrange(
                        "p (b hw) -> p b hw", hw=HW),
                    x_d[b0:b0 + nb, cb * 128:(cb + 1) * 128, :, :]
                    .rearrange("b c h w -> c b (h w)"))
                for k in range(nb):
                    pl = cb * B_SH + b0 + k
                    seg = stage[:, k * HW:(k + 1) * HW]
                    # copy + f32->f32r rounding + pooled sum, one op
                    nc.vector.tensor_scalar(
                        intr(pl, 0, H),
                        seg.rearrange("p (h w) -> p h w", w=W),
                        1.0, 0.0, ALU.mult, ALU.add,
                        accum_out=pooled[:, ccol(pl):ccol(pl) + 1])

            pl_last_flag = [False]  # whether current layer is the last

            def emit_input_and_early_convs(held):
                # DRAM -> stage (paired planes, both HWDGE rings) -> padded
                # layout via DVE tensor_scalar ops that fuse the f32->f32r
                # rounding with the layer-0 pooled accumulation.  Layer-0
                # convs for the first planes are interleaved so the PE works
                # (and its HAM clock-gate stays warm) while input streams in;
                # the budget is SPILL spilled planes + 3 direct PSUM planes.
                # tracked sample-0 pooled seed (host-precomputed sums)
                nc.vector.tensor_copy(pooled[:, B_SH::NC9], p0i_sb[:, :])
                for cb in range(CB):
                    for gi, (b0, nb) in enumerate([(0, 2), (2, 2),
                                                   (4, 2), (6, 2)]):
                        input_group(cb, gi, b0, nb)
                        for k in range(nb):
                            pl = cb * B_SH + b0 + k
                            if pl < SPILL + 3:
                                held.append((pl, emit_conv(pl,
                                                           pl < SPILL)))
                    # layer-0 ih z1 matmul for this cb as its pooled lands
                    nc.tensor.matmul(
                        cellps[0:32, 0:NC9],
                        wih1t_sb[:, cb * 32:(cb + 1) * 32],
                        pooled[:, cb * NC9:(cb + 1) * NC9],
                        start=(cb == 0), stop=(cb == CB - 1))

            def emit_layer(layer, num_layers):
                last = layer == num_layers - 1
                first = layer == 0
                pl_last_flag[0] = last
                held = []
                if first:
                    emit_input_and_early_convs(held)
                    cell_stage(0, True)   # hh-path memset only
                    cell_stage(1, True)
                    cell_stage(2, True)
                    cell_stage(3, True)
                    cell_stage(4, True)
                else:
                    # pooled shard cols = half0 + half1 combine accums
                    pv = pooled[:, :].rearrange("p (cb n) -> p cb n",
                                                n=NC9)[:, :, 0:B_SH]
                    nc.vector.tensor_tensor(
                        pv,
                        poolacc[:, 0:2 * NP:2].rearrange(
                            "p (cb n) -> p cb n", n=B_SH),
                        poolacc[:, 1:2 * NP:2].rearrange(
                            "p (cb n) -> p cb n", n=B_SH),
                        ALU.add)
                    # tracked sample-0 pooled: p0 *= (s_0 + ksum)
                    nc.vector.tensor_tensor(
                        s0k[:, :], s_sb[:, B_SH::NC9], ksum_sb[:, :],
                        ALU.add)
                    nc.vector.tensor_tensor(
                        pooled[:, B_SH::NC9], pooled[:, B_SH::NC9],
                        s0k[:, :], ALU.mult)
                    # interleave the serial cell chain with the first conv
                    # planes (all spilled via ACT, the evicts woven between
                    # the chain's own ACT ops) so neither the PE nor the
                    # PSUM ring ever waits on the chain; combines are
                    # emitted after stage 4 so the dependency binds to THIS
                    # layer's s.
                    held.append((0, emit_conv(0, True)))
                    held.append((1, emit_conv(1, True)))
                    cell_stage(0, False)
                    held.append((2, emit_conv(2, True)))
                    cell_stage(1, False)
                    held.append((3, emit_conv(3, True)))
                    cell_stage(2, False)
                    held.append((4, emit_conv(4, True)))
                    cell_stage(3, False)
                    held.append((5, emit_conv(5, True)))
                    cell_stage(4, False)
                    held.append((6, emit_conv(6, True)))
                    held.append((7, emit_conv(7, True)))
                return held

            def emit_body():
                for layer in range(num_layers):
                    last = layer == num_layers - 1
                    held = emit_layer(layer, num_layers)
                    # DVE-offloaded planes: one generator per plane; their
                    # tap ops get sprinkled between combine emissions so the
                    # in-order DVE fills its PE-wait slices with conv work
                    # (each generator ends by emitting that plane's combine).
                    gens = [dve_tap_ops(q) for q in range(NPE, NP)]

                    def sprinkle(n):
                        # sequential drain: a plane's 9 taps + combine fully
                        # precede the next plane's (the dacc ring plus the
                        # in-order DVE would deadlock on a round-robin)
                        done = 0
                        while done < n and gens:
                            try:
                                next(gens[0])
                                done += 1
                            except StopIteration:
                                gens.pop(0)
                        return done

                    for pl, ch in held:
                        emit_combine(pl, ch, last)
                        sprinkle(2)
                    start = (SPILL + 3) if layer == 0 else SPILL
                    for pl in range(start, NPE):
                        emit_combine(pl, emit_conv(pl, False), last)
                        sprinkle(2)
                    # drain any remaining DVE tap work / combines
                    while gens:
                        sprinkle(4)

            if iters == 1:
                emit_body()
            else:
                with tc.For_i(0, iters, 1):
                    emit_body()

    nc.compile()
    return nc


def prep_inputs(x, w_ih_l1, b_ih_l1, w_ih_l2, b_ih_l2,
                w_hh_l1, b_hh_l1, w_hh_l2, b_hh_l2, dw_kernel):
    """Host-side prep: per-core input maps (weights replicated)."""
    x = np.ascontiguousarray(np.asarray(x, dtype=np.float32))
    diag = np.zeros((CB, 9, 128, 128), np.float32)
    dw = np.asarray(dw_kernel, np.float32).reshape(C, 9)
    idx = np.arange(128)
    for cb in range(CB):
        for t in range(9):
            diag[cb, t, idx, idx] = dw[cb * 128:(cb + 1) * 128, t]
    # l2 weights pre-transposed with the summed gate bias as an extra row
    # (pairs with the ones-row of z1)
    wih2t = np.concatenate(
        [np.asarray(w_ih_l2, np.float32).T,
         (np.asarray(b_ih_l2, np.float32)
          + np.asarray(b_hh_l2, np.float32))[None, :]], axis=0)
    whh2t = np.concatenate(
        [np.asarray(w_hh_l2, np.float32).T,
         np.zeros((1, 3 * C), np.float32)], axis=0)
    common = {
        "diag": diag.reshape(CB * 9 * 128, 128),
        "wih1t": np.ascontiguousarray(
            (np.asarray(w_ih_l1, np.float32) / HW).T),
        "whh1t": np.ascontiguousarray(np.asarray(w_hh_l1, np.float32).T),
        "wih2t": np.ascontiguousarray(wih2t),
        "whh2t": np.ascontiguousarray(whh2t),
        "b1": np.ascontiguousarray(np.stack(
            [np.asarray(b_ih_l1, np.float32),
             np.asarray(b_hh_l1, np.float32)], axis=1)),
        "ksum": np.ascontiguousarray(dw.sum(axis=1).reshape(CB, 128).T),
        "dwv": np.ascontiguousarray(np.concatenate(
            [dw[cb * 128:(cb + 1) * 128, :] for cb in range(CB)], axis=1)),
        "p0init": np.ascontiguousarray(
            x[0].reshape(C, HW).sum(axis=1).reshape(CB, 128).T),
    }
    return [dict(common, x=np.ascontiguousarray(x[i * B_SH:(i + 1) * B_SH]))
            for i in range(N_CORES)]


_cache = {}


def kernel(**inputs) -> np.ndarray:
    num_layers = int(inputs["num_layers"])
    if num_layers not in _cache:
        _cache[num_layers] = build_program(num_layers=num_layers, iters=1)
    nc = _cache[num_layers]
    in_maps = prep_inputs(
        inputs["x"], inputs["w_ih_l1"], inputs["b_ih_l1"], inputs["w_ih_l2"],
        inputs["b_ih_l2"], inputs["w_hh_l1"], inputs["b_hh_l1"],
        inputs["w_hh_l2"], inputs["b_hh_l2"], inputs["dw_kernel"])
    res = run_bass_kernel_spmd(nc, in_maps, list(range(N_CORES)))
    return np.concatenate([res.results[i]["y"] for i in range(N_CORES)],
                          axis=0).astype(np.float32)
